# revision 2
# baseline (speedup 1.0000x reference)
"""ChebConvNet (K=1) Trainium2 kernel: 3x silu(x@W+b) -> logits -> log_softmax.

Sharding: data-parallel over nodes across 8 NeuronCores. x is padded from
200000 to 200704 rows (8 * 25088), each core processes its shard in a
transposed [feat, node] layout so the 128-wide feature dim sits on SBUF
partitions. The last matmul flips back to the natural [node, class] layout
(lhsT = activation tile), and the row-wise log_softmax runs there.

edge_index is unused (ChebConv with K=1 ignores the graph).
"""

import numpy as np

import concourse.bacc as bacc
import concourse.mybir as mybir
import concourse.tile as tile
from concourse.bass_utils import run_bass_kernel_spmd

P = 128          # feature dim == SBUF partitions
C = 40           # classes
N_FULL = 200000
N_CORES = 8
NS = 25088       # nodes per core (padded: 8 * 25088 = 200704)
MACROS = [1024] * 24 + [512]          # phase-A macro tiles (nodes)
BCHUNKS = [32] * 6 + [4]              # phase-B chunks (node groups of 128)
NG = NS // P                          # 196 node groups per core

F32 = mybir.dt.float32
F32R = mybir.dt.float32r
AF = mybir.ActivationFunctionType

_CACHE = {}


def _build():
    if "nc" in _CACHE:
        return _CACHE["nc"]
    nc = bacc.Bacc(None, target_bir_lowering=False)
    xT = nc.declare_dram_parameter("xT", [P, NS], F32, isOutput=False)
    Wd = [nc.declare_dram_parameter(f"W{i}", [P, P], F32, isOutput=False) for i in range(3)]
    W3d = nc.declare_dram_parameter("W3", [P, C], F32, isOutput=False)
    bd = [nc.declare_dram_parameter(f"b{i}", [P, 1], F32, isOutput=False) for i in range(3)]
    b3d = nc.declare_dram_parameter("b3bc", [P, 32 * C], F32, isOutput=False)
    out = nc.declare_dram_parameter("out", [NS, C], F32, isOutput=True)
    out3d = out[:].rearrange("(g p) c -> p g c", p=P)  # [128, 196, 40]

    with tile.TileContext(nc) as tc:
        with (
            tc.tile_pool(name="const", bufs=1) as cpool,
            tc.tile_pool(name="xin", bufs=3) as xin,
            tc.tile_pool(name="hs", bufs=2) as hsp,
            tc.tile_pool(name="zall", bufs=1) as zallp,
            tc.tile_pool(name="pb", bufs=2) as pbp,
            tc.tile_pool(name="ph", bufs=3, space="PSUM") as ph,
            tc.tile_pool(name="pz", bufs=2, space="PSUM") as pz,
        ):
            Wt = []
            for i in range(3):
                w = cpool.tile([P, P], F32R, tag=f"W{i}")
                nc.sync.dma_start(w[:], Wd[i][:].bitcast(F32R))
                Wt.append(w)
            W3t = cpool.tile([P, C], F32R, tag="W3")
            nc.sync.dma_start(W3t[:], W3d[:].bitcast(F32R))
            bt = []
            for i in range(3):
                b = cpool.tile([P, 1], F32, tag=f"b{i}")
                nc.sync.dma_start(b[:], bd[i][:])
                bt.append(b)
            b3t = cpool.tile([P, 32 * C], F32, tag="b3bc")
            nc.sync.dma_start(b3t[:], b3d[:])

            # z' = logits + b3, staged for phase B. [128, 196*40] fp32.
            zall = zallp.tile([P, NG * C], F32, tag="zall")

            # ---- Phase A: matmuls + silu (ACT table set: silu) ----
            n0 = 0
            for mt in MACROS:
                g0, gn = n0 // P, mt // P
                xa = xin.tile([P, 1024], F32R, tag="xa")
                nc.sync.dma_start(
                    xa[:, :mt], xT[:, n0 : n0 + mt].bitcast(F32R)
                )
                h = xa[:, :mt]
                for i in range(3):
                    hp = ph.tile([P, 1024], F32, tag="hpsum")
                    for j in range(0, mt, 512):
                        nc.tensor.matmul(
                            hp[:, j : j + 512],
                            Wt[i][:],
                            h[:, j : j + 512],
                            start=True,
                            stop=True,
                        )
                    hsb = hsp.tile([P, 1024], F32R, tag=f"h{i}")
                    nc.scalar.activation(
                        hsb[:, :mt], hp[:, :mt], AF.Silu, bias=bt[i][:], scale=1.0
                    )
                    h = hsb[:, :mt]
                zp = pz.tile([P, 8 * C], F32, tag="zpsum")
                for g in range(gn):
                    nc.tensor.matmul(
                        zp[:, g * C : (g + 1) * C],
                        h[:, g * P : (g + 1) * P],
                        W3t[:],
                        start=True,
                        stop=True,
                    )
                nc.vector.tensor_add(
                    zall[:, g0 * C : (g0 + gn) * C], zp[:, : gn * C], b3t[:, : gn * C]
                )
                n0 += mt

            # ---- Phase B: exp/ln log_softmax (ACT table set: natural_log_exp) ----
            g0 = 0
            for gn in BCHUNKS:
                zc = zall[:, g0 * C : (g0 + gn) * C]
                e = pbp.tile([P, 32 * C], F32, tag="e")
                nc.scalar.activation(e[:, : gn * C], zc, AF.Exp)
                s = pbp.tile([P, 32], F32, tag="s")
                nc.vector.reduce_sum(
                    s[:, :gn],
                    e[:, : gn * C].rearrange("p (g c) -> p g c", g=gn),
                    axis=mybir.AxisListType.X,
                )
                ls = pbp.tile([P, 32], F32, tag="ls")
                nc.scalar.activation(ls[:, :gn], s[:, :gn], AF.Ln)
                o = pbp.tile([P, 32 * C], F32, tag="o")
                nc.vector.tensor_tensor(
                    o[:, : gn * C].rearrange("p (g c) -> p g c", g=gn),
                    zc.rearrange("p (g c) -> p g c", g=gn),
                    ls[:, :gn].broadcast_to([P, gn, C]),
                    op=mybir.AluOpType.subtract,
                )
                nc.sync.dma_start(
                    out3d[:, g0 : g0 + gn, :],
                    o[:, : gn * C].rearrange("p (g c) -> p g c", g=gn),
                )
                g0 += gn
    nc.compile()
    _CACHE["nc"] = nc
    return nc


def _in_maps(x, W0, b0, W1, b1, W2, b2, W3, b3):
    x = np.asarray(x, dtype=np.float32)
    xpad = np.zeros((N_CORES * NS, P), dtype=np.float32)
    xpad[:N_FULL] = x
    common = {
        "W0": np.asarray(W0, np.float32),
        "W1": np.asarray(W1, np.float32),
        "W2": np.asarray(W2, np.float32),
        "W3": np.asarray(W3, np.float32),
        "b0": np.asarray(b0, np.float32).reshape(P, 1),
        "b1": np.asarray(b1, np.float32).reshape(P, 1),
        "b2": np.asarray(b2, np.float32).reshape(P, 1),
        "b3bc": np.ascontiguousarray(
            np.broadcast_to(np.tile(np.asarray(b3, np.float32), 32), (P, 32 * C))
        ),
    }
    maps = []
    for c in range(N_CORES):
        shard = xpad[c * NS : (c + 1) * NS]
        maps.append({**common, "xT": np.ascontiguousarray(shard.T)})
    return maps


def kernel(**inputs):
    nc = _build()
    maps = _in_maps(
        inputs["x"],
        inputs["W0"], inputs["b0"],
        inputs["W1"], inputs["b1"],
        inputs["W2"], inputs["b2"],
        inputs["W3"], inputs["b3"],
    )
    res = run_bass_kernel_spmd(nc, maps, list(range(N_CORES)))
    out = np.concatenate([res.results[c]["out"] for c in range(N_CORES)], axis=0)
    return out[:N_FULL]


# revision 3
# speedup vs baseline: 1.0443x; 1.0443x over previous
"""ChebConvNet (K=1) Trainium2 kernel: 3x silu(x@W+b) -> logits -> log_softmax.

Sharding: data-parallel over nodes across 8 NeuronCores. x is padded from
200000 to 200704 rows (8 * 25088); each core processes its shard in a
transposed [feat, node] layout so the 128-wide feature dim sits on SBUF
partitions. The last matmul flips back to the natural [node, class] layout
(lhsT = h2 tile in bf16 for fast weight loads), and the row-wise
log_softmax runs there. The device writes output in a partition-major
scratch layout ([128, 196*40] per core); the host unscrambles.

edge_index is unused (ChebConv with K=1 ignores the graph).
"""

import numpy as np

import concourse.bacc as bacc
import concourse.mybir as mybir
import concourse.tile as tile
from concourse.bass_utils import run_bass_kernel_spmd

P = 128          # feature dim == SBUF partitions
C = 40           # classes
N_FULL = 200000
N_CORES = 8
NS = 25088       # nodes per core (padded: 8 * 25088 = 200704)
MACROS = [1024] * 24 + [512]          # phase-A macro tiles (nodes)
BCHUNKS = [32] * 6 + [4]              # phase-B chunks (node groups of 128)
NG = NS // P                          # 196 node groups per core

F32 = mybir.dt.float32
F32R = mybir.dt.float32r
BF16 = mybir.dt.bfloat16
AF = mybir.ActivationFunctionType

_CACHE = {}


def _build():
    if "nc" in _CACHE:
        return _CACHE["nc"]
    nc = bacc.Bacc(None, target_bir_lowering=False)
    xT = nc.declare_dram_parameter("xT", [P, NS], F32, isOutput=False)
    Wd = [nc.declare_dram_parameter(f"W{i}", [P, P], F32, isOutput=False) for i in range(3)]
    W3d = nc.declare_dram_parameter("W3", [P, C], BF16, isOutput=False)
    bd = [nc.declare_dram_parameter(f"b{i}", [P, 1], F32, isOutput=False) for i in range(3)]
    b3d = nc.declare_dram_parameter("b3bc", [P, 32 * C], F32, isOutput=False)
    # partition-major scratch layout; host unscrambles to [NS, C]
    out = nc.declare_dram_parameter("out", [P, NG * C], F32, isOutput=True)

    with tile.TileContext(nc) as tc:
        with (
            tc.tile_pool(name="const", bufs=1) as cpool,
            tc.tile_pool(name="xin", bufs=3) as xin,
            tc.tile_pool(name="hs", bufs=2) as hsp,
            tc.tile_pool(name="zall", bufs=1) as zallp,
            tc.tile_pool(name="pb", bufs=2) as pbp,
            tc.tile_pool(name="ph", bufs=3, space="PSUM") as ph,
            tc.tile_pool(name="pz", bufs=2, space="PSUM") as pz,
        ):
            Wt = []
            for i in range(3):
                w = cpool.tile([P, P], F32R, tag=f"W{i}", name=f"W{i}c")
                nc.sync.dma_start(w[:], Wd[i][:].bitcast(F32R))
                Wt.append(w)
            W3t = cpool.tile([P, C], BF16, tag="W3")
            nc.sync.dma_start(W3t[:], W3d[:])
            bt = []
            for i in range(3):
                b = cpool.tile([P, 1], F32, tag=f"b{i}", name=f"b{i}c")
                nc.sync.dma_start(b[:], bd[i][:])
                bt.append(b)
            b3t = cpool.tile([P, 32 * C], F32, tag="b3bc")
            nc.sync.dma_start(b3t[:], b3d[:])

            # z' = logits + b3 and e = exp(z'), staged for phase B.
            zall = zallp.tile([P, NG * C], F32, tag="zall")
            eall = zallp.tile([P, NG * C], F32, tag="eall")
            sall = zallp.tile([P, NG], F32, tag="sall")
            lsall = zallp.tile([P, NG], F32, tag="lsall")

            # ---- Phase A: matmuls + silu (ACT table set: silu) ----
            n0 = 0
            for mt in MACROS:
                g0, gn = n0 // P, mt // P
                xa = xin.tile([P, 1024], F32R, tag="xa")
                nc.sync.dma_start(
                    xa[:, :mt], xT[:, n0 : n0 + mt].bitcast(F32R)
                )
                h = xa[:, :mt]
                for i in range(3):
                    hp = ph.tile([P, 1024], F32, tag="hpsum")
                    for j in range(0, mt, 512):
                        nc.tensor.matmul(
                            hp[:, j : j + 512],
                            Wt[i][:],
                            h[:, j : j + 512],
                            start=True,
                            stop=True,
                        )
                    hdt = BF16 if i == 2 else F32R
                    hsb = hsp.tile([P, 1024], hdt, tag=f"h{i}", name=f"h{i}t")
                    nc.scalar.activation(
                        hsb[:, :mt], hp[:, :mt], AF.Silu, bias=bt[i][:], scale=1.0
                    )
                    h = hsb[:, :mt]
                zp = pz.tile([P, 8 * C], F32, tag="zpsum")
                for g in range(gn):
                    nc.tensor.matmul(
                        zp[:, g * C : (g + 1) * C],
                        h[:, g * P : (g + 1) * P],
                        W3t[:],
                        start=True,
                        stop=True,
                    )
                nc.vector.tensor_add(
                    zall[:, g0 * C : (g0 + gn) * C], zp[:, : gn * C], b3t[:, : gn * C]
                )
                n0 += mt

            # ---- Phase B: log_softmax (ACT table set: natural_log_exp) ----
            # All Exp ops first, then one Ln — avoids table-set thrashing.
            g0 = 0
            for gn in BCHUNKS:
                nc.scalar.activation(
                    eall[:, g0 * C : (g0 + gn) * C],
                    zall[:, g0 * C : (g0 + gn) * C],
                    AF.Exp,
                )
                nc.vector.reduce_sum(
                    sall[:, g0 : g0 + gn],
                    eall[:, g0 * C : (g0 + gn) * C].rearrange(
                        "p (g c) -> p g c", g=gn
                    ),
                    axis=mybir.AxisListType.X,
                )
                g0 += gn
            nc.scalar.activation(lsall[:], sall[:], AF.Ln)
            g0 = 0
            for gn in BCHUNKS:
                o = pbp.tile([P, 32 * C], F32, tag="o")
                nc.vector.tensor_tensor(
                    o[:, : gn * C].rearrange("p (g c) -> p g c", g=gn),
                    zall[:, g0 * C : (g0 + gn) * C].rearrange(
                        "p (g c) -> p g c", g=gn
                    ),
                    lsall[:, g0 : g0 + gn].broadcast_to([P, gn, C]),
                    op=mybir.AluOpType.subtract,
                )
                nc.sync.dma_start(
                    out[:, g0 * C : (g0 + gn) * C], o[:, : gn * C]
                )
                g0 += gn
    nc.compile()
    _CACHE["nc"] = nc
    return nc


def _in_maps(x, W0, b0, W1, b1, W2, b2, W3, b3):
    import ml_dtypes

    x = np.asarray(x, dtype=np.float32)
    xpad = np.zeros((N_CORES * NS, P), dtype=np.float32)
    xpad[:N_FULL] = x
    common = {
        "W0": np.asarray(W0, np.float32),
        "W1": np.asarray(W1, np.float32),
        "W2": np.asarray(W2, np.float32),
        "W3": np.asarray(W3, np.float32).astype(ml_dtypes.bfloat16),
        "b0": np.asarray(b0, np.float32).reshape(P, 1),
        "b1": np.asarray(b1, np.float32).reshape(P, 1),
        "b2": np.asarray(b2, np.float32).reshape(P, 1),
        "b3bc": np.ascontiguousarray(
            np.broadcast_to(np.tile(np.asarray(b3, np.float32), 32), (P, 32 * C))
        ),
    }
    maps = []
    for c in range(N_CORES):
        shard = xpad[c * NS : (c + 1) * NS]
        maps.append({**common, "xT": np.ascontiguousarray(shard.T)})
    return maps


def _unscramble(res):
    # device out: [128, 196*40] with node = g*128 + p  ->  [25088, 40]
    outs = []
    for c in range(N_CORES):
        o = res.results[c]["out"].reshape(P, NG, C)
        outs.append(np.ascontiguousarray(o.transpose(1, 0, 2)).reshape(NS, C))
    return np.concatenate(outs, axis=0)[:N_FULL]


def kernel(**inputs):
    nc = _build()
    maps = _in_maps(
        inputs["x"],
        inputs["W0"], inputs["b0"],
        inputs["W1"], inputs["b1"],
        inputs["W2"], inputs["b2"],
        inputs["W3"], inputs["b3"],
    )
    res = run_bass_kernel_spmd(nc, maps, list(range(N_CORES)))
    return _unscramble(res)


# revision 4
# speedup vs baseline: 1.1682x; 1.1186x over previous
"""ChebConvNet (K=1) Trainium2 kernel: 3x silu(x@W+b) -> logits -> log_softmax.

Sharding: data-parallel over nodes across 8 NeuronCores. x is padded from
200000 to 200704 rows (8 * 25088); each core processes its shard in a
transposed [feat, node] layout so the 128-wide feature dim sits on SBUF
partitions. The last matmul flips back to the natural [node, class] layout
(lhsT = h2 tile in bf16 for fast weight loads), and the row-wise
log_softmax runs there. The device writes output in a partition-major
scratch layout ([128, 196*40] per core); the host unscrambles.

edge_index is unused (ChebConv with K=1 ignores the graph).
"""

import numpy as np

import concourse.bacc as bacc
import concourse.mybir as mybir
import concourse.tile as tile
from concourse.tile import add_dep_helper
from concourse.bass_utils import run_bass_kernel_spmd

P = 128          # feature dim == SBUF partitions
C = 40           # classes
N_FULL = 200000
N_CORES = 8
NS = 25088       # nodes per core (padded: 8 * 25088 = 200704)
MACROS = [1024] * 24 + [512]          # phase-A macro tiles (nodes)
BCHUNKS = [32] * 6 + [4]              # phase-B chunks (node groups of 128)
NG = NS // P                          # 196 node groups per core

F32 = mybir.dt.float32
F32R = mybir.dt.float32r
BF16 = mybir.dt.bfloat16
AF = mybir.ActivationFunctionType

_CACHE = {}


def _build():
    if "nc" in _CACHE:
        return _CACHE["nc"]
    nc = bacc.Bacc(None, target_bir_lowering=False)
    xT = nc.declare_dram_parameter("xT", [P, NS], BF16, isOutput=False)
    Wd = [nc.declare_dram_parameter(f"W{i}", [P, P], BF16, isOutput=False) for i in range(3)]
    W3d = nc.declare_dram_parameter("W3", [P, C], BF16, isOutput=False)
    bd = [nc.declare_dram_parameter(f"b{i}", [P, 1], F32, isOutput=False) for i in range(3)]
    b3d = nc.declare_dram_parameter("b3bc", [P, 32 * C], F32, isOutput=False)
    # partition-major scratch layout; host unscrambles to [NS, C]
    out = nc.declare_dram_parameter("out", [P, NG * C], F32, isOutput=True)

    with tile.TileContext(nc) as tc:
        with (
            tc.tile_pool(name="const", bufs=1) as cpool,
            tc.tile_pool(name="xin", bufs=3) as xin,
            tc.tile_pool(name="hs", bufs=2) as hsp,
            tc.tile_pool(name="zall", bufs=1) as zallp,
            tc.tile_pool(name="pb", bufs=2) as pbp,
            tc.tile_pool(name="ph", bufs=3, space="PSUM") as ph,
            tc.tile_pool(name="pz", bufs=2, space="PSUM") as pz,
        ):
            Wt = []
            for i in range(3):
                w = cpool.tile([P, P], BF16, tag=f"W{i}", name=f"W{i}c")
                nc.sync.dma_start(w[:], Wd[i][:])
                Wt.append(w)
            W3t = cpool.tile([P, C], BF16, tag="W3")
            nc.sync.dma_start(W3t[:], W3d[:])
            bt = []
            for i in range(3):
                b = cpool.tile([P, 1], F32, tag=f"b{i}", name=f"b{i}c")
                nc.sync.dma_start(b[:], bd[i][:])
                bt.append(b)
            b3t = cpool.tile([P, 32 * C], F32, tag="b3bc")
            nc.sync.dma_start(b3t[:], b3d[:])

            # z' = logits + b3 and e = exp(z'), staged for phase B.
            zall = zallp.tile([P, NG * C], F32, tag="zall")
            eall = zallp.tile([P, NG * C], F32, tag="eall")
            sall = zallp.tile([P, NG], F32, tag="sall")
            lsall = zallp.tile([P, NG], F32, tag="lsall")

            # ---- Phase A: matmuls + silu (ACT table set: silu) ----
            n0 = 0
            for mt in MACROS:
                g0, gn = n0 // P, mt // P
                xa = xin.tile([P, 1024], BF16, tag="xa")
                nc.sync.dma_start(xa[:, :mt], xT[:, n0 : n0 + mt])
                h = xa[:, :mt]
                for i in range(3):
                    hp = ph.tile([P, 1024], F32, tag="hpsum")
                    for j in range(0, mt, 512):
                        nc.tensor.matmul(
                            hp[:, j : j + 512],
                            Wt[i][:],
                            h[:, j : j + 512],
                            start=True,
                            stop=True,
                        )
                    hsb = hsp.tile([P, 1024], BF16, tag=f"h{i}", name=f"h{i}t")
                    last_silu = nc.scalar.activation(
                        hsb[:, :mt], hp[:, :mt], AF.Silu, bias=bt[i][:], scale=1.0
                    )
                    h = hsb[:, :mt]
                zp = pz.tile([P, 8 * C], F32, tag="zpsum")
                for g in range(gn):
                    nc.tensor.matmul(
                        zp[:, g * C : (g + 1) * C],
                        h[:, g * P : (g + 1) * P],
                        W3t[:],
                        start=True,
                        stop=True,
                    )
                nc.vector.tensor_add(
                    zall[:, g0 * C : (g0 + gn) * C], zp[:, : gn * C], b3t[:, : gn * C]
                )
                n0 += mt

            # ---- Phase B: log_softmax (ACT table set: natural_log_exp) ----
            # All Exp ops first, then one Ln — avoids table-set thrashing.
            g0 = 0
            for gn in BCHUNKS:
                exp_i = nc.scalar.activation(
                    eall[:, g0 * C : (g0 + gn) * C],
                    zall[:, g0 * C : (g0 + gn) * C],
                    AF.Exp,
                )
                add_dep_helper(exp_i.ins, last_silu.ins, sync=True,
                               reason="keep exp after all silus (ACT table set)")
                nc.vector.reduce_sum(
                    sall[:, g0 : g0 + gn],
                    eall[:, g0 * C : (g0 + gn) * C].rearrange(
                        "p (g c) -> p g c", g=gn
                    ),
                    axis=mybir.AxisListType.X,
                )
                g0 += gn
            nc.scalar.activation(lsall[:], sall[:], AF.Ln)
            g0 = 0
            for gn in BCHUNKS:
                o = pbp.tile([P, 32 * C], F32, tag="o")
                nc.vector.tensor_tensor(
                    o[:, : gn * C].rearrange("p (g c) -> p g c", g=gn),
                    zall[:, g0 * C : (g0 + gn) * C].rearrange(
                        "p (g c) -> p g c", g=gn
                    ),
                    lsall[:, g0 : g0 + gn].broadcast_to([P, gn, C]),
                    op=mybir.AluOpType.subtract,
                )
                nc.sync.dma_start(
                    out[:, g0 * C : (g0 + gn) * C], o[:, : gn * C]
                )
                g0 += gn
    nc.compile()
    _CACHE["nc"] = nc
    return nc


def _in_maps(x, W0, b0, W1, b1, W2, b2, W3, b3):
    import ml_dtypes

    x = np.asarray(x, dtype=np.float32)
    xpad = np.zeros((N_CORES * NS, P), dtype=ml_dtypes.bfloat16)
    xpad[:N_FULL] = x
    common = {
        "W0": np.asarray(W0, np.float32).astype(ml_dtypes.bfloat16),
        "W1": np.asarray(W1, np.float32).astype(ml_dtypes.bfloat16),
        "W2": np.asarray(W2, np.float32).astype(ml_dtypes.bfloat16),
        "W3": np.asarray(W3, np.float32).astype(ml_dtypes.bfloat16),
        "b0": np.asarray(b0, np.float32).reshape(P, 1),
        "b1": np.asarray(b1, np.float32).reshape(P, 1),
        "b2": np.asarray(b2, np.float32).reshape(P, 1),
        "b3bc": np.ascontiguousarray(
            np.broadcast_to(np.tile(np.asarray(b3, np.float32), 32), (P, 32 * C))
        ),
    }
    maps = []
    for c in range(N_CORES):
        shard = xpad[c * NS : (c + 1) * NS]
        maps.append({**common, "xT": np.ascontiguousarray(shard.T)})
    return maps


def _unscramble(res):
    # device out: [128, 196*40] with node = g*128 + p  ->  [25088, 40]
    outs = []
    for c in range(N_CORES):
        o = res.results[c]["out"].reshape(P, NG, C)
        outs.append(np.ascontiguousarray(o.transpose(1, 0, 2)).reshape(NS, C))
    return np.concatenate(outs, axis=0)[:N_FULL]


def kernel(**inputs):
    nc = _build()
    maps = _in_maps(
        inputs["x"],
        inputs["W0"], inputs["b0"],
        inputs["W1"], inputs["b1"],
        inputs["W2"], inputs["b2"],
        inputs["W3"], inputs["b3"],
    )
    res = run_bass_kernel_spmd(nc, maps, list(range(N_CORES)))
    return _unscramble(res)


# revision 5
# speedup vs baseline: 1.6297x; 1.3951x over previous
"""ChebConvNet (K=1) Trainium2 kernel: 3x silu(x@W+b) -> logits -> log_softmax.

Sharding: data-parallel over nodes across 8 NeuronCores. x is padded from
200000 to 200704 rows (8 * 25088); each core processes its shard in a
transposed [feat, node] layout so the 128-wide feature dim sits on SBUF
partitions. Layers run as separate streaming phases (A0/A1/A2) over the
whole shard so consecutive macro tiles are independent and the scalar
(ACT) engine — the silu bottleneck — stays dense. The last matmul flips
back to the natural [node, class] layout (lhsT = h2 tile), and the
row-wise log_softmax runs there in a final phase (separate ACT table set
for exp/ln). The device writes output partition-major; the host
unscrambles.

edge_index is unused (ChebConv with K=1 ignores the graph).
"""

import numpy as np

import concourse.bacc as bacc
import concourse.mybir as mybir
import concourse.tile as tile
from concourse.tile import add_dep_helper
from concourse.bass_utils import run_bass_kernel_spmd

P = 128          # feature dim == SBUF partitions
C = 40           # classes
N_FULL = 200000
N_CORES = 8
NS = 25088       # nodes per core (padded: 8 * 25088 = 200704)
MT = 1024        # macro tile (nodes) for the layer phases
MACROS = [MT] * 24 + [512]
BCHUNKS = [32] * 6 + [4]              # phase-B chunks (node groups of 128)
NG = NS // P                          # 196 node groups per core

F32 = mybir.dt.float32
BF16 = mybir.dt.bfloat16
AF = mybir.ActivationFunctionType

_CACHE = {}


def _build():
    if "nc" in _CACHE:
        return _CACHE["nc"]
    nc = bacc.Bacc(None, target_bir_lowering=False)
    xT = nc.declare_dram_parameter("xT", [P, NS], BF16, isOutput=False)
    Wd = [nc.declare_dram_parameter(f"W{i}", [P, P], BF16, isOutput=False) for i in range(3)]
    W3d = nc.declare_dram_parameter("W3", [P, C], BF16, isOutput=False)
    bd = [nc.declare_dram_parameter(f"b{i}", [P, 1], F32, isOutput=False) for i in range(3)]
    b3d = nc.declare_dram_parameter("b3bc", [P, 32 * C], F32, isOutput=False)
    # partition-major scratch layout; host unscrambles to [NS, C]
    out = nc.declare_dram_parameter("out", [P, NG * C], F32, isOutput=True)

    with tile.TileContext(nc) as tc:
        with (
            tc.tile_pool(name="const", bufs=1) as cpool,
            tc.tile_pool(name="xin", bufs=3) as xin,
            tc.tile_pool(name="h2s", bufs=2) as h2sp,
            tc.tile_pool(name="big", bufs=1) as bigp,
            tc.tile_pool(name="pb", bufs=2) as pbp,
            tc.tile_pool(name="ph", bufs=3, space="PSUM") as ph,
            tc.tile_pool(name="pz", bufs=2, space="PSUM") as pz,
        ):
            Wt = []
            for i in range(3):
                w = cpool.tile([P, P], BF16, tag=f"W{i}", name=f"W{i}c")
                nc.sync.dma_start(w[:], Wd[i][:])
                Wt.append(w)
            W3t = cpool.tile([P, C], BF16, tag="W3")
            nc.sync.dma_start(W3t[:], W3d[:])
            bt = []
            for i in range(3):
                b = cpool.tile([P, 1], F32, tag=f"b{i}", name=f"b{i}c")
                nc.sync.dma_start(b[:], bd[i][:])
                bt.append(b)
            b3t = cpool.tile([P, 32 * C], F32, tag="b3bc")
            nc.sync.dma_start(b3t[:], b3d[:])

            # whole-shard staging. eall reuses h0's slot (tag "bigA"),
            # which is free once phase A1 has consumed h0.
            h0 = bigp.tile([P, NS], BF16, tag="bigA", name="h0all")
            h1 = bigp.tile([P, NS], BF16, tag="bigB", name="h1all")
            zall = bigp.tile([P, NG * C], F32, tag="zall")
            sall = bigp.tile([P, NG], F32, tag="sall")
            lsall = bigp.tile([P, NG], F32, tag="lsall")

            # ---- Phase A0: h0 = silu(x @ W0 + b0) ----
            n0 = 0
            for mt in MACROS:
                xa = xin.tile([P, MT], BF16, tag="xa")
                nc.sync.dma_start(xa[:, :mt], xT[:, n0 : n0 + mt])
                hp = ph.tile([P, MT], F32, tag="hpsum", name="hp0")
                for j in range(0, mt, 512):
                    nc.tensor.matmul(
                        hp[:, j : j + 512], Wt[0][:], xa[:, j : j + 512],
                        start=True, stop=True,
                    )
                nc.scalar.activation(
                    h0[:, n0 : n0 + mt], hp[:, :mt], AF.Silu,
                    bias=bt[0][:], scale=1.0,
                )
                n0 += mt

            # ---- Phase A1: h1 = silu(h0 @ W1 + b1) ----
            n0 = 0
            for mt in MACROS:
                hp = ph.tile([P, MT], F32, tag="hpsum", name="hp1")
                for j in range(0, mt, 512):
                    nc.tensor.matmul(
                        hp[:, j : j + 512], Wt[1][:], h0[:, n0 + j : n0 + j + 512],
                        start=True, stop=True,
                    )
                nc.scalar.activation(
                    h1[:, n0 : n0 + mt], hp[:, :mt], AF.Silu,
                    bias=bt[1][:], scale=1.0,
                )
                n0 += mt

            # ---- Phase A2: h2 = silu(h1 @ W2 + b2); z = h2 @ W3 + b3 ----
            n0 = 0
            for mt in MACROS:
                g0, gn = n0 // P, mt // P
                hp = ph.tile([P, MT], F32, tag="hpsum", name="hp2")
                for j in range(0, mt, 512):
                    nc.tensor.matmul(
                        hp[:, j : j + 512], Wt[2][:], h1[:, n0 + j : n0 + j + 512],
                        start=True, stop=True,
                    )
                h2 = h2sp.tile([P, MT], BF16, tag="h2")
                last_silu = nc.scalar.activation(
                    h2[:, :mt], hp[:, :mt], AF.Silu, bias=bt[2][:], scale=1.0
                )
                zp = pz.tile([P, 8 * C], F32, tag="zpsum")
                for g in range(gn):
                    nc.tensor.matmul(
                        zp[:, g * C : (g + 1) * C],
                        h2[:, g * P : (g + 1) * P],
                        W3t[:],
                        start=True, stop=True,
                    )
                nc.vector.tensor_add(
                    zall[:, g0 * C : (g0 + gn) * C], zp[:, : gn * C], b3t[:, : gn * C]
                )
                n0 += mt

            # ---- Phase B: log_softmax (ACT table set: natural_log_exp) ----
            # eall reuses h0's SBUF slot; all Exp before the single Ln.
            eall = bigp.tile([P, NG * C], F32, tag="bigA", name="eall")
            g0 = 0
            for gn in BCHUNKS:
                exp_i = nc.scalar.activation(
                    eall[:, g0 * C : (g0 + gn) * C],
                    zall[:, g0 * C : (g0 + gn) * C],
                    AF.Exp,
                )
                add_dep_helper(exp_i.ins, last_silu.ins, sync=True,
                               reason="exp after all silus (ACT table set)")
                nc.vector.reduce_sum(
                    sall[:, g0 : g0 + gn],
                    eall[:, g0 * C : (g0 + gn) * C].rearrange(
                        "p (g c) -> p g c", g=gn
                    ),
                    axis=mybir.AxisListType.X,
                )
                g0 += gn
            nc.scalar.activation(lsall[:], sall[:], AF.Ln)
            g0 = 0
            for gn in BCHUNKS:
                o = pbp.tile([P, 32 * C], F32, tag="o")
                nc.vector.tensor_tensor(
                    o[:, : gn * C].rearrange("p (g c) -> p g c", g=gn),
                    zall[:, g0 * C : (g0 + gn) * C].rearrange(
                        "p (g c) -> p g c", g=gn
                    ),
                    lsall[:, g0 : g0 + gn].broadcast_to([P, gn, C]),
                    op=mybir.AluOpType.subtract,
                )
                nc.sync.dma_start(
                    out[:, g0 * C : (g0 + gn) * C], o[:, : gn * C]
                )
                g0 += gn
    nc.compile()
    _CACHE["nc"] = nc
    return nc


def _in_maps(x, W0, b0, W1, b1, W2, b2, W3, b3):
    import ml_dtypes

    x = np.asarray(x, dtype=np.float32)
    xpad = np.zeros((N_CORES * NS, P), dtype=ml_dtypes.bfloat16)
    xpad[:N_FULL] = x
    common = {
        "W0": np.asarray(W0, np.float32).astype(ml_dtypes.bfloat16),
        "W1": np.asarray(W1, np.float32).astype(ml_dtypes.bfloat16),
        "W2": np.asarray(W2, np.float32).astype(ml_dtypes.bfloat16),
        "W3": np.asarray(W3, np.float32).astype(ml_dtypes.bfloat16),
        "b0": np.asarray(b0, np.float32).reshape(P, 1),
        "b1": np.asarray(b1, np.float32).reshape(P, 1),
        "b2": np.asarray(b2, np.float32).reshape(P, 1),
        "b3bc": np.ascontiguousarray(
            np.broadcast_to(np.tile(np.asarray(b3, np.float32), 32), (P, 32 * C))
        ),
    }
    maps = []
    for c in range(N_CORES):
        shard = xpad[c * NS : (c + 1) * NS]
        maps.append({**common, "xT": np.ascontiguousarray(shard.T)})
    return maps


def _unscramble(res):
    # device out: [128, 196*40] with node = g*128 + p  ->  [25088, 40]
    outs = []
    for c in range(N_CORES):
        o = res.results[c]["out"].reshape(P, NG, C)
        outs.append(np.ascontiguousarray(o.transpose(1, 0, 2)).reshape(NS, C))
    return np.concatenate(outs, axis=0)[:N_FULL]


def kernel(**inputs):
    nc = _build()
    maps = _in_maps(
        inputs["x"],
        inputs["W0"], inputs["b0"],
        inputs["W1"], inputs["b1"],
        inputs["W2"], inputs["b2"],
        inputs["W3"], inputs["b3"],
    )
    res = run_bass_kernel_spmd(nc, maps, list(range(N_CORES)))
    return _unscramble(res)


# revision 6
# speedup vs baseline: 1.6831x; 1.0328x over previous
"""ChebConvNet (K=1) Trainium2 kernel: 3x silu(x@W+b) -> logits -> log_softmax.

Sharding: data-parallel over nodes across 8 NeuronCores. x is padded from
200000 to 200704 rows (8 * 25088); each core processes its shard in a
transposed [feat, node] layout so the 128-wide feature dim sits on SBUF
partitions. Layers run as separate streaming phases (A0/A1/A2) over the
whole shard so consecutive macro tiles are independent and the scalar
(ACT) engine — the silu bottleneck — stays dense. The last matmul flips
back to the natural [node, class] layout (lhsT = h2 tile), and the
row-wise log_softmax runs there in a final phase (separate ACT table set
for exp/ln). The device writes output partition-major; the host
unscrambles.

edge_index is unused (ChebConv with K=1 ignores the graph).
"""

import numpy as np

import concourse.bacc as bacc
import concourse.mybir as mybir
import concourse.tile as tile
from concourse.tile import add_dep_helper
from concourse.bass_utils import run_bass_kernel_spmd

P = 128          # feature dim == SBUF partitions
C = 40           # classes
N_FULL = 200000
N_CORES = 8
NS = 25088       # nodes per core (padded: 8 * 25088 = 200704)
MT = 1024        # macro tile (nodes) for the layer phases
MACROS = [MT] * 24 + [512]
BCHUNKS = [32] * 6 + [4]              # phase-B chunks (node groups of 128)
NG = NS // P                          # 196 node groups per core

F32 = mybir.dt.float32
BF16 = mybir.dt.bfloat16
AF = mybir.ActivationFunctionType

_CACHE = {}


def _build():
    if "nc" in _CACHE:
        return _CACHE["nc"]
    nc = bacc.Bacc(None, target_bir_lowering=False)
    xT = nc.declare_dram_parameter("xT", [P, NS], BF16, isOutput=False)
    CB = 3 * 2 * P + 2 * C + 3 * 4 + 4 * 32 * C  # 5980 bytes/partition
    cd = nc.declare_dram_parameter("consts", [P, CB], mybir.dt.uint8, isOutput=False)
    # partition-major scratch layout; host unscrambles to [NS, C]
    out = nc.declare_dram_parameter("out", [P, NG * C], F32, isOutput=True)

    with tile.TileContext(nc) as tc:
        with (
            tc.tile_pool(name="const", bufs=1) as cpool,
            tc.tile_pool(name="xin", bufs=3) as xin,
            tc.tile_pool(name="h2s", bufs=2) as h2sp,
            tc.tile_pool(name="big", bufs=1) as bigp,
            tc.tile_pool(name="pb", bufs=3) as pbp,
            tc.tile_pool(name="ph", bufs=3, space="PSUM") as ph,
            tc.tile_pool(name="pz", bufs=2, space="PSUM") as pz,
        ):
            craw = cpool.tile([P, CB], mybir.dt.uint8, tag="craw")
            nc.sync.dma_start(craw[:], cd[:])
            off = 0
            Wt = []
            for i in range(3):
                Wt.append(craw[:, off : off + 2 * P].bitcast(BF16))
                off += 2 * P
            W3t = craw[:, off : off + 2 * C].bitcast(BF16)
            off += 2 * C
            bt = []
            for i in range(3):
                bt.append(craw[:, off : off + 4].bitcast(F32))
                off += 4
            b3t = craw[:, off : off + 4 * 32 * C].bitcast(F32)

            # whole-shard staging. eall reuses h0's slot (tag "bigA"),
            # which is free once phase A1 has consumed h0.
            h0 = bigp.tile([P, NS], BF16, tag="bigA", name="h0all")
            h1 = bigp.tile([P, NS], BF16, tag="bigB", name="h1all")
            zall = bigp.tile([P, NG * C], F32, tag="zall")
            sall = bigp.tile([P, NG], F32, tag="sall")
            lsall = bigp.tile([P, NG], F32, tag="lsall")

            # ---- Phase A0: h0 = silu(x @ W0 + b0) ----
            n0 = 0
            for mt in MACROS:
                xa = xin.tile([P, MT], BF16, tag="xa")
                nc.sync.dma_start(xa[:, :mt], xT[:, n0 : n0 + mt])
                hp = ph.tile([P, MT], F32, tag="hpsum", name="hp0")
                for j in range(0, mt, 512):
                    nc.tensor.matmul(
                        hp[:, j : j + 512], Wt[0], xa[:, j : j + 512],
                        start=True, stop=True,
                    )
                nc.scalar.activation(
                    h0[:, n0 : n0 + mt], hp[:, :mt], AF.Silu,
                    bias=bt[0], scale=1.0,
                )
                n0 += mt

            # ---- Phase A1: h1 = silu(h0 @ W1 + b1) ----
            n0 = 0
            for mt in MACROS:
                hp = ph.tile([P, MT], F32, tag="hpsum", name="hp1")
                for j in range(0, mt, 512):
                    nc.tensor.matmul(
                        hp[:, j : j + 512], Wt[1], h0[:, n0 + j : n0 + j + 512],
                        start=True, stop=True,
                    )
                nc.scalar.activation(
                    h1[:, n0 : n0 + mt], hp[:, :mt], AF.Silu,
                    bias=bt[1], scale=1.0,
                )
                n0 += mt

            # ---- Phase A2: h2 = silu(h1 @ W2 + b2); z = h2 @ W3 + b3 ----
            n0 = 0
            for mt in MACROS:
                g0, gn = n0 // P, mt // P
                hp = ph.tile([P, MT], F32, tag="hpsum", name="hp2")
                for j in range(0, mt, 512):
                    nc.tensor.matmul(
                        hp[:, j : j + 512], Wt[2], h1[:, n0 + j : n0 + j + 512],
                        start=True, stop=True,
                    )
                h2 = h2sp.tile([P, MT], BF16, tag="h2")
                last_silu = nc.scalar.activation(
                    h2[:, :mt], hp[:, :mt], AF.Silu, bias=bt[2], scale=1.0
                )
                zp = pz.tile([P, 8 * C], F32, tag="zpsum")
                for g in range(gn):
                    nc.tensor.matmul(
                        zp[:, g * C : (g + 1) * C],
                        h2[:, g * P : (g + 1) * P],
                        W3t,
                        start=True, stop=True,
                    )
                nc.vector.tensor_add(
                    zall[:, g0 * C : (g0 + gn) * C], zp[:, : gn * C], b3t[:, : gn * C]
                )
                n0 += mt

            # ---- Phase B: log_softmax (ACT table set: natural_log_exp) ----
            # eall reuses h0's SBUF slot; all Exp before the single Ln.
            eall = bigp.tile([P, NG * C], F32, tag="bigA", name="eall")
            g0 = 0
            for k, gn in enumerate(BCHUNKS):
                exp_i = nc.scalar.activation(
                    eall[:, g0 * C : (g0 + gn) * C],
                    zall[:, g0 * C : (g0 + gn) * C],
                    AF.Exp,
                )
                add_dep_helper(exp_i.ins, last_silu.ins, sync=True,
                               reason="exp after all silus (ACT table set)")
                nc.vector.reduce_sum(
                    sall[:, g0 : g0 + gn],
                    eall[:, g0 * C : (g0 + gn) * C].rearrange(
                        "p (g c) -> p g c", g=gn
                    ),
                    axis=mybir.AxisListType.X,
                )
                nc.scalar.activation(
                    lsall[:, g0 : g0 + gn], sall[:, g0 : g0 + gn], AF.Ln
                )
                o = pbp.tile([P, 32 * C], F32, tag="o")
                sub_engine = nc.gpsimd if k % 2 == 0 else nc.vector
                sub_engine.tensor_tensor(
                    o[:, : gn * C].rearrange("p (g c) -> p g c", g=gn),
                    zall[:, g0 * C : (g0 + gn) * C].rearrange(
                        "p (g c) -> p g c", g=gn
                    ),
                    lsall[:, g0 : g0 + gn].broadcast_to([P, gn, C]),
                    op=mybir.AluOpType.subtract,
                )
                nc.sync.dma_start(
                    out[:, g0 * C : (g0 + gn) * C], o[:, : gn * C]
                )
                g0 += gn
    nc.compile()
    _CACHE["nc"] = nc
    return nc


def _in_maps(x, W0, b0, W1, b1, W2, b2, W3, b3):
    import ml_dtypes

    x = np.asarray(x, dtype=np.float32)
    xpad = np.zeros((N_CORES * NS, P), dtype=ml_dtypes.bfloat16)
    xpad[:N_FULL] = x
    parts = [
        np.asarray(W0, np.float32).astype(ml_dtypes.bfloat16).view(np.uint8),
        np.asarray(W1, np.float32).astype(ml_dtypes.bfloat16).view(np.uint8),
        np.asarray(W2, np.float32).astype(ml_dtypes.bfloat16).view(np.uint8),
        np.asarray(W3, np.float32).astype(ml_dtypes.bfloat16).view(np.uint8),
        np.asarray(b0, np.float32).reshape(P, 1).view(np.uint8),
        np.asarray(b1, np.float32).reshape(P, 1).view(np.uint8),
        np.asarray(b2, np.float32).reshape(P, 1).view(np.uint8),
        np.ascontiguousarray(
            np.broadcast_to(np.tile(np.asarray(b3, np.float32), 32), (P, 32 * C))
        ).view(np.uint8),
    ]
    common = {"consts": np.ascontiguousarray(np.concatenate(parts, axis=1))}
    maps = []
    for c in range(N_CORES):
        shard = xpad[c * NS : (c + 1) * NS]
        maps.append({**common, "xT": np.ascontiguousarray(shard.T)})
    return maps


def _unscramble(res):
    # device out: [128, 196*40] with node = g*128 + p  ->  [25088, 40]
    outs = []
    for c in range(N_CORES):
        o = res.results[c]["out"].reshape(P, NG, C)
        outs.append(np.ascontiguousarray(o.transpose(1, 0, 2)).reshape(NS, C))
    return np.concatenate(outs, axis=0)[:N_FULL]


def kernel(**inputs):
    nc = _build()
    maps = _in_maps(
        inputs["x"],
        inputs["W0"], inputs["b0"],
        inputs["W1"], inputs["b1"],
        inputs["W2"], inputs["b2"],
        inputs["W3"], inputs["b3"],
    )
    res = run_bass_kernel_spmd(nc, maps, list(range(N_CORES)))
    return _unscramble(res)


# revision 7
# speedup vs baseline: 1.7351x; 1.0309x over previous
"""ChebConvNet (K=1) Trainium2 kernel: 3x silu(x@W+b) -> logits -> log_softmax.

Sharding: data-parallel over nodes across 8 NeuronCores. x is padded from
200000 to 200704 rows (8 * 25088); each core processes its shard in a
transposed [feat, node] layout so the 128-wide feature dim sits on SBUF
partitions. Layers run as separate streaming phases (A0/A1/A2) over the
whole shard so consecutive macro tiles are independent and the scalar
(ACT) engine — the silu bottleneck — stays dense. The last matmul flips
back to the natural [node, class] layout (lhsT = h2 tile), and the
row-wise log_softmax runs there in a final phase (separate ACT table set
for exp/ln). The device writes output partition-major; the host
unscrambles.

edge_index is unused (ChebConv with K=1 ignores the graph).
"""

import numpy as np

import concourse.bacc as bacc
import concourse.mybir as mybir
import concourse.tile as tile
from concourse.tile import add_dep_helper
from concourse.bass_utils import run_bass_kernel_spmd

P = 128          # feature dim == SBUF partitions
C = 40           # classes
N_FULL = 200000
N_CORES = 8
NS = 25088       # nodes per core (padded: 8 * 25088 = 200704)
MT = 1536        # macro tile (nodes) for phases A0/A1
MACROS = [MT] * 16 + [512]               # 16*1536 + 512 = 25088
MACROS2 = [1024] * 24 + [512]            # phase A2 (psum shared with zp pool)
BCHUNKS = [32] * 6 + [4]              # phase-B chunks (node groups of 128)
NG = NS // P                          # 196 node groups per core

F32 = mybir.dt.float32
BF16 = mybir.dt.bfloat16
AF = mybir.ActivationFunctionType

_CACHE = {}


def _build():
    if "nc" in _CACHE:
        return _CACHE["nc"]
    nc = bacc.Bacc(None, target_bir_lowering=False)
    xT = nc.declare_dram_parameter("xT", [P, NS], BF16, isOutput=False)
    CB = 3 * 2 * P + 2 * C + 3 * 4 + 4 * 32 * C  # 5980 bytes/partition
    cd = nc.declare_dram_parameter("consts", [P, CB], mybir.dt.uint8, isOutput=False)
    # partition-major scratch layout; host unscrambles to [NS, C]
    out = nc.declare_dram_parameter("out", [P, NG * C], F32, isOutput=True)

    with tile.TileContext(nc) as tc:
        with (
            tc.tile_pool(name="const", bufs=1) as cpool,
            tc.tile_pool(name="xin", bufs=3) as xin,
            tc.tile_pool(name="h2s", bufs=2) as h2sp,
            tc.tile_pool(name="big", bufs=1) as bigp,
            tc.tile_pool(name="pb", bufs=3) as pbp,
            tc.tile_pool(name="ph", bufs=2, space="PSUM") as ph,
            tc.tile_pool(name="pz", bufs=2, space="PSUM") as pz,
        ):
            craw = cpool.tile([P, CB], mybir.dt.uint8, tag="craw")
            nc.sync.dma_start(craw[:], cd[:])
            off = 0
            Wt = []
            for i in range(3):
                Wt.append(craw[:, off : off + 2 * P].bitcast(BF16))
                off += 2 * P
            W3t = craw[:, off : off + 2 * C].bitcast(BF16)
            off += 2 * C
            bt = []
            for i in range(3):
                bt.append(craw[:, off : off + 4].bitcast(F32))
                off += 4
            b3t = craw[:, off : off + 4 * 32 * C].bitcast(F32)

            # whole-shard staging. eall reuses h0's slot (tag "bigA"),
            # which is free once phase A1 has consumed h0.
            h0 = bigp.tile([P, NS], BF16, tag="bigA", name="h0all")
            h1 = bigp.tile([P, NS], BF16, tag="bigB", name="h1all")
            zall = bigp.tile([P, NG * C], F32, tag="zall")
            sall = bigp.tile([P, NG], F32, tag="sall")
            lsall = bigp.tile([P, NG], F32, tag="lsall")

            # ---- Phase A0: h0 = silu(x @ W0 + b0) ----
            n0 = 0
            for mt in MACROS:
                xa = xin.tile([P, MT], BF16, tag="xa")
                nc.sync.dma_start(xa[:, :mt], xT[:, n0 : n0 + mt])
                hp = ph.tile([P, MT], F32, tag="hpsum", name="hp0")
                for j in range(0, mt, 512):
                    nc.tensor.matmul(
                        hp[:, j : j + 512], Wt[0], xa[:, j : j + 512],
                        start=True, stop=True,
                    )
                nc.scalar.activation(
                    h0[:, n0 : n0 + mt], hp[:, :mt], AF.Silu,
                    bias=bt[0], scale=1.0,
                )
                n0 += mt

            # ---- Phase A1: h1 = silu(h0 @ W1 + b1) ----
            n0 = 0
            for mt in MACROS:
                hp = ph.tile([P, MT], F32, tag="hpsum", name="hp1")
                for j in range(0, mt, 512):
                    nc.tensor.matmul(
                        hp[:, j : j + 512], Wt[1], h0[:, n0 + j : n0 + j + 512],
                        start=True, stop=True,
                    )
                nc.scalar.activation(
                    h1[:, n0 : n0 + mt], hp[:, :mt], AF.Silu,
                    bias=bt[1], scale=1.0,
                )
                n0 += mt

            # ---- Phase A2: h2 = silu(h1 @ W2 + b2); z = h2 @ W3 + b3 ----
            n0 = 0
            for mt in MACROS2:
                g0, gn = n0 // P, mt // P
                hp = ph.tile([P, 1024], F32, tag="hpsum", name="hp2")
                for j in range(0, mt, 512):
                    nc.tensor.matmul(
                        hp[:, j : j + 512], Wt[2], h1[:, n0 + j : n0 + j + 512],
                        start=True, stop=True,
                    )
                h2 = h2sp.tile([P, 1024], BF16, tag="h2")
                last_silu = nc.scalar.activation(
                    h2[:, :mt], hp[:, :mt], AF.Silu, bias=bt[2], scale=1.0
                )
                zp = pz.tile([P, 8 * C], F32, tag="zpsum")
                for g in range(gn):
                    nc.tensor.matmul(
                        zp[:, g * C : (g + 1) * C],
                        h2[:, g * P : (g + 1) * P],
                        W3t,
                        start=True, stop=True,
                    )
                nc.vector.tensor_add(
                    zall[:, g0 * C : (g0 + gn) * C], zp[:, : gn * C], b3t[:, : gn * C]
                )
                n0 += mt

            # ---- Phase B: log_softmax (ACT table set: natural_log_exp) ----
            # eall reuses h0's SBUF slot; all Exp before the single Ln.
            eall = bigp.tile([P, NG * C], F32, tag="bigA", name="eall")
            g0 = 0
            for k, gn in enumerate(BCHUNKS):
                exp_i = nc.scalar.activation(
                    eall[:, g0 * C : (g0 + gn) * C],
                    zall[:, g0 * C : (g0 + gn) * C],
                    AF.Exp,
                )
                add_dep_helper(exp_i.ins, last_silu.ins, sync=True,
                               reason="exp after all silus (ACT table set)")
                nc.vector.reduce_sum(
                    sall[:, g0 : g0 + gn],
                    eall[:, g0 * C : (g0 + gn) * C].rearrange(
                        "p (g c) -> p g c", g=gn
                    ),
                    axis=mybir.AxisListType.X,
                )
                nc.scalar.activation(
                    lsall[:, g0 : g0 + gn], sall[:, g0 : g0 + gn], AF.Ln
                )
                o = pbp.tile([P, 32 * C], F32, tag="o")
                sub_engine = nc.gpsimd if k < 3 else nc.vector
                sub_engine.tensor_tensor(
                    o[:, : gn * C].rearrange("p (g c) -> p g c", g=gn),
                    zall[:, g0 * C : (g0 + gn) * C].rearrange(
                        "p (g c) -> p g c", g=gn
                    ),
                    lsall[:, g0 : g0 + gn].broadcast_to([P, gn, C]),
                    op=mybir.AluOpType.subtract,
                )
                nc.sync.dma_start(
                    out[:, g0 * C : (g0 + gn) * C], o[:, : gn * C]
                )
                g0 += gn
    nc.compile()
    _CACHE["nc"] = nc
    return nc


def _in_maps(x, W0, b0, W1, b1, W2, b2, W3, b3):
    import ml_dtypes

    x = np.asarray(x, dtype=np.float32)
    xpad = np.zeros((N_CORES * NS, P), dtype=ml_dtypes.bfloat16)
    xpad[:N_FULL] = x
    parts = [
        np.asarray(W0, np.float32).astype(ml_dtypes.bfloat16).view(np.uint8),
        np.asarray(W1, np.float32).astype(ml_dtypes.bfloat16).view(np.uint8),
        np.asarray(W2, np.float32).astype(ml_dtypes.bfloat16).view(np.uint8),
        np.asarray(W3, np.float32).astype(ml_dtypes.bfloat16).view(np.uint8),
        np.asarray(b0, np.float32).reshape(P, 1).view(np.uint8),
        np.asarray(b1, np.float32).reshape(P, 1).view(np.uint8),
        np.asarray(b2, np.float32).reshape(P, 1).view(np.uint8),
        np.ascontiguousarray(
            np.broadcast_to(np.tile(np.asarray(b3, np.float32), 32), (P, 32 * C))
        ).view(np.uint8),
    ]
    common = {"consts": np.ascontiguousarray(np.concatenate(parts, axis=1))}
    maps = []
    for c in range(N_CORES):
        shard = xpad[c * NS : (c + 1) * NS]
        maps.append({**common, "xT": np.ascontiguousarray(shard.T)})
    return maps


def _unscramble(res):
    # device out: [128, 196*40] with node = g*128 + p  ->  [25088, 40]
    outs = []
    for c in range(N_CORES):
        o = res.results[c]["out"].reshape(P, NG, C)
        outs.append(np.ascontiguousarray(o.transpose(1, 0, 2)).reshape(NS, C))
    return np.concatenate(outs, axis=0)[:N_FULL]


def kernel(**inputs):
    nc = _build()
    maps = _in_maps(
        inputs["x"],
        inputs["W0"], inputs["b0"],
        inputs["W1"], inputs["b1"],
        inputs["W2"], inputs["b2"],
        inputs["W3"], inputs["b3"],
    )
    res = run_bass_kernel_spmd(nc, maps, list(range(N_CORES)))
    return _unscramble(res)
